# revision 41
# baseline (speedup 1.0000x reference)
"""Trainium2 Bass kernel for a top-2 MoE layer (B=2, T=2048, D=1024, F=4096, E=8).

Strategy (expert-parallel, per sharding hint):
  Launch 1 (router, data-parallel over tokens): each of 8 cores computes
    logits = x_slice @ Wr with BOTH operands split bf16 hi+lo (~17
    mantissa bits each side — beyond the fp22 precision the PE's fp32
    path truncates the moving operand to, so top-2 selection is
    flip-free; verified against exact fp64 on the fixed inputs).
    All-bf16 matmuls ride FWL; per (token-tile, d-tile) two stationary
    x loads stream against the [Wr_hi | Wr_lo] moving blocks, and one
    DVE add folds the two logit column blocks. Top-2 combine weights
    come from the sigmoid identity p1/(p1+p2) = sigmoid(l1-l2).
    x rides the sync HWDGE queue (the fast one: ~0.35 B/ns/core; the
    scalar HWDGE queue measures ~3x slower, gpsimd is software-DGE and
    slower still), Wr + comb writes ride scalar.
  Host dispatch (data movement only): tokens are gathered per expert
    (the all-to-all is performed by the host), padded to a static
    capacity.
  Launch 2 (expert FFN, expert-parallel): core e holds expert e's W1/W2
    fully resident in SBUF (bf16: 128 KiB/partition), computes
    y = gelu(x@W1+b1)@W2 + b2 for its gathered tokens. All bulk input
    (x chunks, W1 f-slabs, W2 slabs) rides the sync queue in exact
    consumption order; chunk emission order s1c0, s1c1, s2c0, s1c2,
    s2c1, s2c2 keeps stage 2 off the W2 stream's critical path; y is
    written once, in bf16, on gpsimd.
  Launch 3 (combine): out[t] = ca*A[t] + cb*B[t] — the two selected
    experts' outputs per token (bf16), combined on-device. Four big
    pieces (each DMA trigger costs ~650ns of queue-engine time, so many
    small pieces go trigger-bound), a/b/o on sync, weights on scalar.

All arithmetic is on-device; the host only reshapes/gathers/concats.
"""

import numpy as np

import concourse.bacc as bacc
import concourse.mybir as mybir
import concourse.tile as tile
from concourse import bass_utils

F32 = mybir.dt.float32
F32R = mybir.dt.float32r
BF16 = mybir.dt.bfloat16
NPBF16 = mybir.dt.np(mybir.dt.bfloat16)
AX = mybir.AxisListType
ALU = mybir.AluOpType
ACT_F = mybir.ActivationFunctionType

B, T, D, F, E = 2, 2048, 1024, 4096, 8
NTOK = B * T              # 4096
NCORES = 8
TOK_PER_CORE = NTOK // NCORES  # 512
DO = D // 128             # 8 d-tiles
FT = F // 128             # 32 f-tiles

_cache = {}


def _run(nc, in_maps, trace=False, **kw):
    return bass_utils.run_bass_kernel_spmd(
        nc, in_maps, core_ids=list(range(NCORES)), trace=trace, **kw
    )


# ----------------------------------------------------------------- router ---
def build_router():
    """Per core: xT2 [128, 4, 2, 8, 128] bf16 (hi/lo split of x:
    xT2[p,tt,h,do,c] = hi/lo(x[tt*128+c, do*128+p]); hi+lo carries ~17
    mantissa bits so the logits match the fp32 GEMM to ~1e-5),
    Wr2 [128, 8, 2, 8] bf16 (hi|lo column blocks) -> comb [128, 4, 8] fp32
    (comb[p,tt,e] for token tt*128+p). All-bf16 matmuls: LDWEIGHTS rides
    FWL, each (tt,do) is 2 stationary loads x N=16 moving; the two 8-wide
    logit column blocks (x@Wr_hi, x@Wr_lo) are summed by one vector add."""
    if "router" in _cache:
        return _cache["router"]
    TT = TOK_PER_CORE // 128  # 4 token tiles
    nc = bacc.Bacc("TRN2", target_bir_lowering=False, debug=False)
    xT_d = nc.dram_tensor("xT_sl", [128, TT * 2 * DO * 128], BF16,
                          kind="ExternalInput").ap()
    wr_d = nc.dram_tensor("Wr2", [128, DO * 2 * E], BF16,
                          kind="ExternalInput").ap()
    out_d = nc.dram_tensor("comb", [128, TT * E], F32, kind="ExternalOutput").ap()
    xT_ap = xT_d.rearrange("p (t h o c) -> p t h o c", t=TT, h=2, o=DO)

    with tile.TileContext(nc) as tc:
        with (
            tc.tile_pool(name="pool", bufs=1) as pool,
            tc.tile_pool(name="work", bufs=2) as work,
            tc.tile_pool(name="lgp", bufs=4, space="PSUM") as lgp,
            tc.tile_pool(name="psw", bufs=1, space="PSUM") as psw,
        ):
            xT_sb = pool.tile([128, TT, 2, DO, 128], BF16)
            wr_sb = pool.tile([128, DO, 2, E], BF16)
            comb_sb = pool.tile([128, TT, E], F32)
            warm_sb = pool.tile([128, 128], BF16)
            dum = pool.tile([1, 1], F32)
            dum2 = pool.tile([1, 1], F32)

            # Sigmoid table preload + PE p-state ramp during x DMA.
            # Short warm MMs pace the PE through the HAM window with no
            # >3.4us idle gap before the first logit matmul.
            nc.gpsimd.memset(dum[:], 0.0)
            nc.scalar.activation(dum2[:], dum[:], ACT_F.Sigmoid)
            nc.gpsimd.memset(warm_sb[:], 0.0)
            warm_ps = psw.tile([128, 128], F32)
            for _ in range(14):
                nc.tensor.matmul(warm_ps[:], warm_sb[:], warm_sb[:],
                                 start=True, stop=True)

            # All of x rides sync (the fast HWDGE queue; the scalar HWDGE
            # queue measures ~3x slower and gpsimd is software-DGE), as one
            # transfer per token tile, tt-major. Wr goes first on scalar
            # (small, needed by the very first matmul); comb writes also
            # ride scalar so they never queue behind x.
            nc.scalar.dma_start(
                wr_sb[:], wr_d.rearrange("p (o h e) -> p o h e", o=DO, h=2))
            for tt in range(TT):
                for h in range(2):
                    nc.sync.dma_start(xT_sb[:, tt, h], xT_ap[:, tt, h])

            def emit_mm(tt):
                # lg [tok, E] += xhi@wrhi + xhi@wrlo + xlo@wrhi (the lo*lo
                # cross term is ~2^-18 relative — dropped). All hi/lo
                # products accumulate into the SAME PSUM columns, so the
                # top-2 chain reads the logits straight out of PSUM with no
                # block-fold DVE ops.
                lg = lgp.tile([128, E], F32, tag="lg")
                n = 0
                for do in range(DO):
                    for hx, hw in ((0, 0), (0, 1), (1, 0)):
                        nc.tensor.matmul(
                            lg[:], xT_sb[:, tt, hx, do, :],
                            wr_sb[:, do, hw],
                            start=(n == 0), stop=(n == 3 * DO - 1),
                        )
                        n += 1
                return lg

            def emit_chain(tt, l, v):
                mx1 = work.tile([128, 1], F32, tag="mx1")
                v.reduce_max(mx1[:], l[:], axis=AX.X)
                eq1 = work.tile([128, E], F32, tag="eq1")
                v.tensor_scalar(eq1[:], l[:], mx1[:], None, op0=ALU.is_equal)
                lm = work.tile([128, E], F32, tag="lm")
                v.scalar_tensor_tensor(
                    lm[:], eq1[:], -1e30, l[:], op0=ALU.mult, op1=ALU.add
                )
                mx2 = work.tile([128, 1], F32, tag="mx2")
                v.reduce_max(mx2[:], lm[:], axis=AX.X)
                # s2 = sigmoid(mx2 - mx1), with the subtraction folded into
                # the ACT engine's scale*in + bias preamble
                s2 = work.tile([128, 1], F32, tag="s2")
                nc.scalar.activation(s2[:], mx1[:], ACT_F.Sigmoid,
                                     bias=mx2[:], scale=-1.0)
                s1m2 = work.tile([128, 1], F32, tag="s1m2")
                v.tensor_scalar(s1m2[:], s2[:], -2.0, 1.0,
                                op0=ALU.mult, op1=ALU.add)
                ge = work.tile([128, E], F32, tag="ge")
                v.tensor_scalar(ge[:], l[:], mx2[:], None, op0=ALU.is_ge)
                t1 = work.tile([128, E], F32, tag="t1")
                v.tensor_scalar_mul(t1[:], ge[:], s2[:])
                v.scalar_tensor_tensor(
                    comb_sb[:, tt, :], eq1[:], s1m2[:], t1[:],
                    op0=ALU.mult, op1=ALU.add,
                )

            # Per tt: MMs -> block add -> top-2 chain -> comb write.
            out_ap = out_d.rearrange("p (t e) -> p t e", t=TT)
            for tt in range(TT):
                l = emit_mm(tt)
                emit_chain(tt, l, nc.vector)
                nc.scalar.dma_start(out_ap[:, tt], comb_sb[:, tt])
    nc.compile()
    _cache["router"] = nc
    return nc


# -------------------------------------------------------------------- ffn ---
W1_SLABS = [(0, 1), (1, 2), (2, 4), (4, 6), (6, 8), (8, 12), (12, 16),
            (16, 24), (24, 32)]
W2_SLABS = [(0, 2), (2, 4), (4, 8), (8, 16), (16, 24), (24, 32)]


def build_ffn(cap, with_b1=False, with_b2=False):
    """Per core (expert e), all bf16 except biases:
    xTg [128, DO*cap], W1e [128, FT*DO*128], W2e [128, FT*D],
    b1e [128, FT] f32, b2e/ones bf16
    -> y [128, DO*ttiles*128] bf16, yT[p, dt, t] = y[token t, d = dt*128+p].

    W1+W2 fully SBUF-resident; all bulk input DMA rides the sync queue in
    consumption order (xc0, W1 slabs, xc1, W2 slabs, xc2, ...); chunk
    emission order s1c0, s1c1, s2c0, s1c2, s2c1, ... keeps the PE off the
    W2 stream's critical path."""
    key = ("ffn", cap, with_b1, with_b2)
    if key in _cache:
        return _cache[key]
    assert cap % 32 == 0
    TTILES = -(-cap // 128)
    chunks = _chunk_split(cap)
    CHUNKMAX = max(cs for _, cs in chunks)
    NCH = len(chunks)

    nc = bacc.Bacc("TRN2", target_bir_lowering=False, debug=False)
    xT_d = nc.dram_tensor("xTg", [128, DO * cap], BF16, kind="ExternalInput").ap()
    w1_d = nc.dram_tensor("W1e", [128, FT * DO * 128], BF16,
                          kind="ExternalInput").ap()
    w2_d = nc.dram_tensor("W2e", [128, FT * D], BF16, kind="ExternalInput").ap()
    if with_b1:
        b1_d = nc.dram_tensor("b1e", [128, FT], F32, kind="ExternalInput").ap()
    if with_b2:
        b2_d = nc.dram_tensor("b2e", [1, D], BF16, kind="ExternalInput").ap()
        ones_d = nc.dram_tensor("ones", [1, 512], BF16, kind="ExternalInput").ap()
    y_d = nc.dram_tensor("y", [128, DO * TTILES * 128], BF16,
                         kind="ExternalOutput").ap()
    w1_ap = w1_d.rearrange("p (f o c) -> p f o c", f=FT, o=DO)
    w2_ap = w2_d.rearrange("p (f d) -> p f d", f=FT)
    y_ap = y_d.rearrange("p (o t) -> p o t", o=DO)

    with tile.TileContext(nc) as tc:
        with (
            tc.tile_pool(name="res", bufs=1) as res,
            tc.tile_pool(name="xtp", bufs=2) as xtp,
            tc.tile_pool(name="htp", bufs=2) as htp,
            tc.tile_pool(name="ysp", bufs=3) as ysp,
            tc.tile_pool(name="ps1", bufs=3, space="PSUM") as ps1,
            tc.tile_pool(name="ps2", bufs=2, space="PSUM") as ps2,
        ):
            w1_sb = res.tile([128, FT, DO, 128], BF16)   # 64 KiB/partition
            w2_sb = res.tile([128, FT, D], BF16)         # 64 KiB/partition
            if with_b1:
                b1_sb = res.tile([128, FT], F32)
            if with_b2:
                b2_sb = res.tile([1, D], BF16)
                ones_sb = res.tile([1, 512], BF16)
            warm_sb = res.tile([128, CHUNKMAX], BF16)
            gdum = res.tile([1, 1], BF16)

            # Gelu table preload + PE ramp while the first transfers land
            # (~9 cold warm MMs end right as x chunk 0 + W1 f0 arrive).
            nc.gpsimd.memset(warm_sb[:], 0.0)
            nc.scalar.activation(gdum[:], warm_sb[:1, :1], ACT_F.Gelu)
            for _ in range(8):
                warm_ps = ps1.tile([128, CHUNKMAX], F32, tag="hp")
                nc.tensor.matmul(warm_ps[:], warm_sb[:, :128], warm_sb[:],
                                 start=True, stop=True)

            # --- bulk input DMA: everything latency-critical (x chunk 0,
            # W1 slabs, W2 slabs) rides sync, the fast HWDGE queue, in
            # consumption order. Later x chunks ride gpsimd (software DGE,
            # slow but they have tens of microseconds of slack); the scalar
            # queue stays DMA-free so gelu ACTIVATEs never queue behind
            # 650ns DMA triggers.
            xts = [None] * NCH

            def load_xt(ci, q):
                c0, cs = chunks[ci]
                xT_sb = xtp.tile([128, DO, CHUNKMAX], BF16, tag="xt")
                off = DO * c0
                src = xT_d[:, off:off + DO * cs].rearrange("p (o t) -> p o t", o=DO)
                q.dma_start(xT_sb[:, :, :cs], src)
                xts[ci] = xT_sb

            load_xt(0, nc.sync)
            for lo, hi in W1_SLABS:
                nc.sync.dma_start(w1_sb[:, lo:hi], w1_ap[:, lo:hi])
            if NCH > 1:
                load_xt(1, nc.sync)
            for lo, hi in W2_SLABS:
                nc.sync.dma_start(w2_sb[:, lo:hi], w2_ap[:, lo:hi])
            for ci in range(2, NCH):
                load_xt(ci, nc.sync)
            if with_b1:
                nc.gpsimd.dma_start(b1_sb[:], b1_d[:])
            if with_b2:
                nc.gpsimd.dma_start(b2_sb[:], b2_d[:])
                nc.gpsimd.dma_start(ones_sb[:], ones_d[:])

            hts = [None] * NCH

            def stage1(ci):
                c0, cs = chunks[ci]
                xT_sb = xts[ci]
                hT_sb = htp.tile([128, FT, CHUNKMAX], BF16, tag="ht")
                for ft in range(FT):
                    hp = ps1.tile([128, CHUNKMAX], F32, tag="hp")
                    for do in range(DO):
                        nc.tensor.matmul(
                            hp[:, :cs], w1_sb[:, ft, do, :], xT_sb[:, do, :cs],
                            start=(do == 0), stop=(do == DO - 1),
                        )
                    if with_b1:
                        nc.scalar.activation(hT_sb[:, ft, :cs], hp[:, :cs],
                                             ACT_F.Gelu, bias=b1_sb[:, ft:ft + 1])
                    else:
                        nc.scalar.activation(hT_sb[:, ft, :cs], hp[:, :cs],
                                             ACT_F.Gelu)
                hts[ci] = hT_sb

            def stage2(ci):
                c0, cs = chunks[ci]
                hT_sb = hts[ci]
                for dt in range(DO):
                    yp = ps2.tile([128, CHUNKMAX], F32, tag="yp")
                    for fo in range(FT):
                        nc.tensor.matmul(
                            yp[:, :cs],
                            w2_sb[:, fo, dt * 128:(dt + 1) * 128],
                            hT_sb[:, fo, :cs],
                            start=(fo == 0),
                            stop=(fo == FT - 1 and not with_b2),
                        )
                    if with_b2:
                        nc.tensor.matmul(
                            yp[:, :cs],
                            b2_sb[:, dt * 128:(dt + 1) * 128],
                            ones_sb[:, :cs],
                            start=False, stop=True,
                        )
                    y_sb = ysp.tile([128, CHUNKMAX], BF16, tag="y")
                    nc.vector.tensor_copy(y_sb[:, :cs], yp[:, :cs])
                    nc.sync.dma_start(y_ap[:, dt, c0:c0 + cs], y_sb[:, :cs])

            # s1c0, s1c1, s2c0, s1c2, s2c1, ..., s2c(last)
            stage1(0)
            if NCH > 1:
                stage1(1)
            stage2(0)
            for ci in range(2, NCH):
                stage1(ci)
                stage2(ci - 1)
            if NCH > 1:
                stage2(NCH - 1)
    nc.compile()
    _cache[key] = nc
    return nc


# ---------------------------------------------------------------- combine ---
def build_combine():
    """Per core: packed a, b [128, (T/128)*D] bf16 (raw expert outputs) and
    per-token weights ca, cb [128, 4] f32 -> o = ca*a + cb*b bf16.

    Host packs A[t, d] -> Ah[p, tt*D + d] with t = tt*128 + p; tokens sit on
    partitions, so the combine weights are per-partition scalars. a/b pieces
    interleave on the sync queue; o writes ride gpsimd."""
    if "comb" in _cache:
        return _cache["comb"]
    TT = TOK_PER_CORE // 128  # 4
    W = TT * D  # 4096
    NP = 4  # pieces (few big pieces: each DMA trigger costs ~650ns of
    #                 queue-engine time, so many small pieces go trigger-bound)
    PW = W // NP
    nc = bacc.Bacc("TRN2", target_bir_lowering=False, debug=False)
    a_d = nc.dram_tensor("a", [128, W], BF16, kind="ExternalInput").ap()
    b_d = nc.dram_tensor("b", [128, W], BF16, kind="ExternalInput").ap()
    cab_d = nc.dram_tensor("cab", [128, 2 * TT], F32, kind="ExternalInput").ap()
    o_d = nc.dram_tensor("o", [128, W], BF16, kind="ExternalOutput").ap()
    with tile.TileContext(nc) as tc:
        with (
            tc.tile_pool(name="res", bufs=1) as res,
            tc.tile_pool(name="pool", bufs=4) as pool,
        ):
            cab_sb = res.tile([128, 2 * TT], F32)
            nc.scalar.dma_start(cab_sb[:], cab_d[:])
            ca_sb = cab_sb[:, :TT]
            cb_sb = cab_sb[:, TT:]
            tiles = []
            for pc in range(NP):
                sl = slice(pc * PW, (pc + 1) * PW)
                at = pool.tile([128, PW], BF16, tag="a")
                bt = pool.tile([128, PW], BF16, tag="b")
                nc.sync.dma_start(at[:], a_d[:, sl])
                nc.sync.dma_start(bt[:], b_d[:, sl])
                tiles.append((at, bt))
            for pc in range(NP):
                sl = slice(pc * PW, (pc + 1) * PW)
                tti = pc * PW // D
                at, bt = tiles[pc]
                wt = pool.tile([128, PW], BF16, tag="w")
                ot = pool.tile([128, PW], BF16, tag="o")
                nc.vector.tensor_scalar_mul(wt[:], bt[:], cb_sb[:, tti:tti + 1])
                nc.vector.scalar_tensor_tensor(
                    ot[:], at[:], ca_sb[:, tti:tti + 1], wt[:],
                    op0=ALU.mult, op1=ALU.add,
                )
                nc.sync.dma_start(o_d[:, sl], ot[:])
    nc.compile()
    _cache["comb"] = nc
    return nc


# ----------------------------------------------------------------- driver ---
def _chunk_split(cap):
    """Split cap (multiple of 32) into chunks: chunk0 ~320 for an early PE
    start while W1 streams, the rest ~balanced <=448 (PSUM bank limit 512).
    All sizes multiples of 32."""
    assert cap % 32 == 0
    if cap <= 512:
        return [(0, cap)]
    c0 = 320
    rem = cap - c0
    k = -(-rem // 448)
    base = rem // k // 32 * 32
    sizes = [c0] + [base + 32 if i < (rem - base * k) // 32 else base
                    for i in range(k)]
    chunks, off = [], 0
    for cs in sizes:
        chunks.append((off, cs))
        off += cs
    assert off == cap
    return chunks


def _moe_forward(x2d, Wr, W1, b1, W2, b2, trace=False):
    """x2d: [NTOK, D] fp32. Returns (out [NTOK, D] fp32, exec_ns_total|None)."""
    TT = TOK_PER_CORE // 128

    # --- launch 1: router ---
    rnc = build_router()
    wrp = Wr.reshape(DO, 128, E).transpose(1, 0, 2)   # [128, DO, E] fp32
    wr_hi = wrp.astype(NPBF16)
    wr_lo = (wrp - wr_hi.astype(np.float32)).astype(NPBF16)
    wrh = np.ascontiguousarray(
        np.stack([wr_hi, wr_lo], axis=2).reshape(128, -1))

    def pack_x(c):
        A = x2d[c * TOK_PER_CORE:(c + 1) * TOK_PER_CORE].reshape(
            TT, 128, DO, 128)
        hi = A.astype(NPBF16)
        lo = (A - hi.astype(np.float32)).astype(NPBF16)
        S = np.stack([hi, lo], axis=1)          # [TT, 2, u, DO, v]
        return np.ascontiguousarray(
            S.transpose(4, 0, 1, 3, 2).reshape(128, -1))

    in_maps = [{"xT_sl": pack_x(c), "Wr2": wrh} for c in range(NCORES)]
    rres = _run(rnc, in_maps, trace=trace)
    comb = np.concatenate(
        [rres.results[c]["comb"].reshape(128, TT, E)
         .transpose(1, 0, 2).reshape(TOK_PER_CORE, E) for c in range(NCORES)],
        axis=0)
    global _last_comb
    _last_comb = comb
    exec_ns = rres.exec_time_ns or 0
    per_launch = [rres.exec_time_ns]

    # --- host dispatch (data movement only) ---
    top2 = np.argpartition(-comb, 1, axis=1)[:, :2]  # [NTOK, 2]
    sel_lists, cvals = [], []
    for e in range(E):
        sel = np.nonzero((top2 == e).any(axis=1))[0]
        sel_lists.append(sel)
        cvals.append(comb[sel, e])
    counts = np.array([len(s) for s in sel_lists])
    MAXCAP = 3072
    nbatch = max(1, -(-int(counts.max()) // MAXCAP))
    cap = int(max(256, -(-(-(-counts.max() // nbatch)) // 32) * 32))

    fnc = build_ffn(cap, with_b1=bool(np.any(b1)), with_b2=bool(np.any(b2)))
    chunks = _chunk_split(cap)
    ttiles = -(-cap // 128)
    ones_in = np.ones((1, 512), NPBF16)
    x2d_bf = x2d.astype(NPBF16)
    w_packed = [
        {"W1e": np.ascontiguousarray(
            W1[e].reshape(DO, 128, FT, 128).transpose(1, 2, 0, 3)
            .reshape(128, -1).astype(NPBF16)),
         "W2e": np.ascontiguousarray(
            W2[e].reshape(FT, 128, D).transpose(1, 0, 2)
            .reshape(128, -1).astype(NPBF16))}
        for e in range(E)
    ]
    if np.any(b1):
        for e in range(E):
            w_packed[e]["b1e"] = np.ascontiguousarray(b1[e].reshape(FT, 128).T)
    if np.any(b2):
        for e in range(E):
            w_packed[e]["b2e"] = np.ascontiguousarray(
                b2[e].astype(NPBF16)).reshape(1, D)
            w_packed[e]["ones"] = ones_in
    ys = [np.zeros((0, D), NPBF16) for _ in range(E)]
    for bi in range(nbatch):
        in_maps = []
        for e in range(E):
            sel_b = sel_lists[e][bi * cap:(bi + 1) * cap]
            n_e = len(sel_b)
            xsel = np.zeros((cap, D), NPBF16)
            xsel[:n_e] = x2d_bf[sel_b]
            xg = np.concatenate(
                [xsel[c0:c0 + cs].reshape(cs, DO, 128).transpose(2, 1, 0)
                 .reshape(128, -1) for (c0, cs) in chunks], axis=1)
            in_maps.append({"xTg": np.ascontiguousarray(xg), **w_packed[e]})
        fres = _run(fnc, in_maps, trace=trace)
        # y arrives transposed: [128, DO, ttiles*128] with y[t, dt*128+p]
        ys = [np.concatenate([
            ys[e],
            fres.results[e]["y"].reshape(128, DO, ttiles * 128)
            .transpose(2, 1, 0).reshape(ttiles * 128, D)[:cap]])
            for e in range(E)]
        exec_ns += fres.exec_time_ns or 0
        per_launch.append(fres.exec_time_ns)

    # --- host: build per-token (A, B) contribution rows (gather only) ---
    slot = np.zeros((NTOK, E), np.int64)
    for e in range(E):
        slot[sel_lists[e], e] = np.arange(counts[e])
    e1, e2v = top2[:, 0], top2[:, 1]
    A = np.empty((NTOK, D), NPBF16)
    Bm = np.empty((NTOK, D), NPBF16)
    for e in range(E):
        m1 = e1 == e
        A[m1] = ys[e][slot[m1, e]]
        m2 = e2v == e
        Bm[m2] = ys[e][slot[m2, e]]

    # --- launch 3: combine ---
    cnc = build_combine()

    def pack(m, c):
        sl = m[c * TOK_PER_CORE:(c + 1) * TOK_PER_CORE]
        return np.ascontiguousarray(
            sl.reshape(TOK_PER_CORE // 128, 128, D).transpose(1, 0, 2)
            .reshape(128, -1))

    ca = comb[np.arange(NTOK), e1].astype(np.float32)
    cb = comb[np.arange(NTOK), e2v].astype(np.float32)

    def packc(v, c):
        sl = v[c * TOK_PER_CORE:(c + 1) * TOK_PER_CORE]
        return np.ascontiguousarray(sl.reshape(TOK_PER_CORE // 128, 128).T)

    in_maps = [{"a": pack(A, c), "b": pack(Bm, c),
                "cab": np.ascontiguousarray(
                    np.hstack([packc(ca, c), packc(cb, c)]))}
               for c in range(NCORES)]
    cres = _run(cnc, in_maps, trace=trace)
    out = np.concatenate(
        [cres.results[c]["o"].astype(np.float32)
         .reshape(128, TOK_PER_CORE // 128, D)
         .transpose(1, 0, 2).reshape(TOK_PER_CORE, D) for c in range(NCORES)],
        axis=0)
    exec_ns += cres.exec_time_ns or 0
    per_launch.append(cres.exec_time_ns)
    if trace:
        print(f"per-launch exec ns (router, ffn, combine): {per_launch}")
        _moe_forward.last = (rres, fres, cres)
    return out, (exec_ns if trace else None)


def kernel(x, Wr, W1, b1, W2, b2):
    x = np.asarray(x, np.float32)
    out, _ = _moe_forward(
        x.reshape(NTOK, D),
        np.asarray(Wr, np.float32),
        np.asarray(W1, np.float32),
        np.asarray(b1, np.float32),
        np.asarray(W2, np.float32),
        np.asarray(b2, np.float32),
        trace=False,
    )
    return out.reshape(B, T, D)
